# revision 22
# baseline (speedup 1.0000x reference)
"""Multi-head attention + residual + LayerNorm on 8 Trainium2 NeuronCores.

Sharding: core c in 0..7 handles batch b = c//4 and query-row quarter
r = c%4 (rows 512r..512r+512 of S=2048), with ALL 16 heads.  key/value
are replicated to every core (host-side staging); each core computes the
full-sequence K^T and V projections itself — measured collectives on this
stack cost ~130us per 2MB AllGather, far more than the ~70us of redundant
PE work, and the local pipeline keeps the PE clock warm.

Per core:
  - transpose x tiles on PE (fp32r, via identity), project:
      Q^T [1024, 512] (own rows),  K^T [1024, 2048] -> local DRAM,
      V [2048, 8, 130] pair-blocks with ones columns -> local DRAM
  - per head pair p, per sk chunk c: S^T = K_h Q_h^T  (PSUM) -> exp
    (ACT, scale 1/8) -> U^T accumulation with lhsT = V_aug; the ones
    column makes row 64 the softmax denominators
  - normalize: reciprocal of sums row, PE K=1 broadcast to 64
    partitions, multiply -> ctx^T [128, 8, 512]
  - out = ctx @ Wo + bo + residual -> LayerNorm -> y rows [512, 1024]

All matmuls in float32r (full-rate fp32 PE path, ~2e-4 rel err).
"""

import sys

if "/opt/trn_rl_repo" not in sys.path:
    sys.path.insert(0, "/opt/trn_rl_repo")

import numpy as np

import concourse.bacc as bacc
import concourse.bass as bass
import concourse.mybir as mybir
import concourse.tile as tile
from concourse.bass import ds, ts
from concourse.bass_utils import run_bass_kernel_spmd

FP32R = mybir.dt.float32r
FP32 = mybir.dt.float32
AF = mybir.ActivationFunctionType
ALU = mybir.AluOpType

N_CORES = 8
B = 2
S = 2048
D = 1024
H = 16
DK = 64
P = 128

SL = S // 4  # 512 local query rows per core
KC = D // P  # 8 contraction chunks over d_model
SQ = SL // P  # 4 sq subchunks of 128 (per 512-row block)
CH = S // P  # 16 sk chunks
PAIRS = H // 2  # 8 head pairs
NB = 4  # 512-row blocks of the full sequence
EPS = 1e-5

_NC_CACHE = {}


def build_nc():
    nc = bacc.Bacc(num_devices=N_CORES)

    xq_d = nc.dram_tensor("xq", [SL, D], FP32R, kind="ExternalInput")
    xk_d = nc.dram_tensor("xk", [S, D], FP32R, kind="ExternalInput")
    xv_d = nc.dram_tensor("xv", [S, D], FP32R, kind="ExternalInput")
    wq_d = nc.dram_tensor("wq", [D, D], FP32R, kind="ExternalInput")
    wk_d = nc.dram_tensor("wk", [D, D], FP32R, kind="ExternalInput")
    wv_d = nc.dram_tensor("wv", [D, D], FP32R, kind="ExternalInput")
    wo_d = nc.dram_tensor("wo", [D, D], FP32R, kind="ExternalInput")
    bq_d = nc.dram_tensor("bq", [D], FP32, kind="ExternalInput")
    bk_d = nc.dram_tensor("bk", [D], FP32, kind="ExternalInput")
    bv_d = nc.dram_tensor("bv", [D], FP32, kind="ExternalInput")
    bo_d = nc.dram_tensor("bo", [D], FP32, kind="ExternalInput")
    gam_d = nc.dram_tensor("gam", [D], FP32, kind="ExternalInput")
    bet_d = nc.dram_tensor("bet", [D], FP32, kind="ExternalInput")
    ident_d = nc.dram_tensor("ident", [P, P], FP32R, kind="ExternalInput")
    ones_d = nc.dram_tensor("ones", [P, 64], FP32R, kind="ExternalInput")

    y_d = nc.dram_tensor("y", [SL, D], FP32, kind="ExternalOutput")

    # local DRAM for the full-sequence K^T and augmented V
    kt_d = nc.dram_tensor("ktf", [D, S], FP32R)
    vf_d = nc.dram_tensor("vf", [S, PAIRS, 130], FP32R)

    with tile.TileContext(nc) as tc:
        with (
            tc.tile_pool(name="consts", bufs=1) as consts,
            tc.tile_pool(name="big", bufs=1) as big,
            tc.tile_pool(name="xtp", bufs=1) as xtp,
            tc.tile_pool(name="wide", bufs=2) as wide,
            tc.tile_pool(name="xnp", bufs=4) as xnp,
            tc.tile_pool(name="wpool", bufs=1) as wpool,
            tc.tile_pool(name="stream", bufs=2) as stream,
            tc.tile_pool(name="kttp", bufs=7) as kttp,
            tc.tile_pool(name="etp", bufs=4) as etp,
            tc.tile_pool(name="vat", bufs=2) as vatp,
            tc.tile_pool(name="small", bufs=2) as small,
            tc.tile_pool(name="psA", bufs=3, space="PSUM") as psA,
            tc.tile_pool(name="psAcc", bufs=2, space="PSUM") as psAcc,
            tc.tile_pool(name="psB", bufs=1, space="PSUM") as psB,
        ):
            # ---- constants ----
            ident = consts.tile([P, P], FP32R)
            nc.sync.dma_start(ident[:], ident_d[:])
            ones64 = consts.tile([P, 64], FP32R)
            nc.sync.dma_start(ones64[:], ones_d[:])
            bq_sb = consts.tile([P, KC], FP32)
            nc.sync.dma_start(bq_sb[:], bq_d.rearrange("(m q) -> q m", q=P))
            bk_sb = consts.tile([P, KC], FP32)
            nc.sync.dma_start(bk_sb[:], bk_d.rearrange("(m q) -> q m", q=P))

            def bcast_load(src, tag):
                t = consts.tile([P, D], FP32, tag=tag)
                ap = bass.AP(tensor=src, offset=0, ap=[[0, P], [1, D]])
                nc.gpsimd.dma_start(out=t[:], in_=ap)
                return t

            bv_b = bcast_load(bv_d, "bv_b")
            bo_b = bcast_load(bo_d, "bo_b")
            gam_b = bcast_load(gam_d, "gam_b")
            bet_b = bcast_load(bet_d, "bet_b")
            eps_t = consts.tile([P, 1], FP32)
            nc.vector.memset(eps_t[:], EPS)

            def load_xT(x_d, row0):
                """x rows [row0:row0+512] -> x^T SBUF [128, KC, 512]."""
                xT = xtp.tile([P, KC, SL], FP32R, tag="xT")
                for i in range(SQ):
                    xt = xnp.tile([P, D], FP32R, tag="xnat")
                    nc.sync.dma_start(xt[:], x_d[ds(row0 + i * P, P), :])
                    for j in range(KC):
                        pt = psA.tile([P, P], FP32R, tag="mm")
                        nc.tensor.transpose(pt[:], xt[:, ts(j, P)], ident[:])
                        nc.vector.tensor_copy(xT[:, j, ts(i, P)], pt[:])
                return xT

            # ---- K^T full sequence -> kt_d, block by block ----
            kt_dr = kt_d.rearrange("(m q) s -> q m s", q=P)
            wk_sb = wpool.tile([P, KC, D], FP32R, tag="wrhs")
            for k in range(KC):
                nc.sync.dma_start(wk_sb[:, k, :], wk_d[ts(k, P), :])
            for blk in range(NB):
                xkT = load_xT(xk_d, blk * SL)
                for m in range(KC):
                    pp = psA.tile([P, SL], FP32, tag="mm")
                    for k in range(KC):
                        nc.tensor.matmul(
                            pp[:],
                            wk_sb[:, k, ts(m, P)],
                            xkT[:, k, :],
                            start=(k == 0),
                            stop=(k == KC - 1),
                        )
                    kev = stream.tile([P, SL], FP32R, tag="kev")
                    nc.scalar.activation(
                        out=kev[:],
                        in_=pp[:],
                        func=AF.Identity,
                        bias=bk_sb[:, m : m + 1],
                    )
                    nc.sync.dma_start(kt_dr[:, m, ds(blk * SL, SL)], kev[:])

            # ---- V full sequence -> vf_d (pair-augmented layout) ----
            vf_dr = vf_d.rearrange("(i q) p e -> q i p e", q=P)
            wv_sb = wpool.tile([P, KC, D], FP32R, tag="wrhs")
            for k in range(KC):
                nc.sync.dma_start(wv_sb[:, k, :], wv_d[ts(k, P), :])
            for blk in range(NB):
                xvT = load_xT(xv_d, blk * SL)
                for n in range(2):
                    for i in range(SQ):
                        pp = psA.tile([P, 512], FP32, tag="mm")
                        for k in range(KC):
                            nc.tensor.matmul(
                                pp[:],
                                xvT[:, k, ts(i, P)],
                                wv_sb[:, k, ds(n * 512, 512)],
                                start=(k == 0),
                                stop=(k == KC - 1),
                            )
                        # vtmp holds [4 pairs x (V_even |1| V_odd |1)] = 520 cols
                        vtmp = stream.tile([P, 4, 130], FP32R, tag="vtmp")
                        vdst = vtmp[:].rearrange("q pl (j e) -> q pl j e", e=65)
                        nc.vector.tensor_tensor(
                            vdst[:, :, :, 0:64],
                            pp[:].rearrange("q (pl j e) -> q pl j e", pl=4, j=2),
                            bv_b[:, ds(n * 512, 512)].rearrange(
                                "q (pl j e) -> q pl j e", pl=4, j=2
                            ),
                            ALU.add,
                        )
                        nc.vector.tensor_copy(
                            vdst[:, :, :, 64:65], ones64[:, 0:8, None].rearrange(
                                "q (pl j) o -> q pl j o", pl=4
                            )
                        )
                        ii = blk * SQ + i
                        nc.sync.dma_start(vf_dr[:, ii, ds(n * 4, 4), :], vtmp[:])

            # ---- Q^T (own rows) ----
            xqT = load_xT(xq_d, 0)
            qt_sb = big.tile([P, KC, SL], FP32R, tag="qt")
            wq_sb = wpool.tile([P, KC, D], FP32R, tag="wrhs")
            for k in range(KC):
                nc.sync.dma_start(wq_sb[:, k, :], wq_d[ts(k, P), :])
            for m in range(KC):
                pp = psA.tile([P, SL], FP32, tag="mm")
                for k in range(KC):
                    nc.tensor.matmul(
                        pp[:],
                        wq_sb[:, k, ts(m, P)],
                        xqT[:, k, :],
                        start=(k == 0),
                        stop=(k == KC - 1),
                    )
                nc.scalar.activation(
                    out=qt_sb[:, m, :],
                    in_=pp[:],
                    func=AF.Identity,
                    bias=bq_sb[:, m : m + 1],
                )

            # ---- attention ----
            ctx_sb = big.tile([P, PAIRS, SL], FP32R, tag="ctx")
            vf_blk = vf_d.rearrange("(b i q) p e -> q b i p e", b=NB, q=P)

            def emit_normalize(np_, uA, uB):
                # rows 0..63 of ut / row 64 -> ctx_sb[:, np_, :]
                for j, ut in enumerate((uA, uB)):
                    rec = small.tile([P, SL], FP32R, tag="rec")
                    with nc.allow_low_precision(
                        reason="float32r is bit-identical to float32 in SBUF"
                    ):
                        nc.vector.reciprocal(out=rec[64:65, :], in_=ut[64:65, :])
                    bc = psB.tile([P, SL], FP32, tag="bc")
                    nc.tensor.matmul(
                        bc[0:64, :],
                        ones64[64:65, :],
                        rec[64:65, :],
                        start=True,
                        stop=True,
                    )
                    bc_sb = small.tile([P, SL], FP32, tag="bcs")
                    nc.vector.tensor_copy(bc_sb[0:64, :], bc[0:64, :])
                    if j == 0:
                        nc.vector.tensor_tensor(
                            ctx_sb[0:64, np_, :], ut[0:64, :], bc_sb[0:64, :], ALU.mult
                        )
                    else:
                        ctmp = small.tile([P, SL], FP32R, tag="ctmp")
                        nc.vector.tensor_tensor(
                            ctmp[0:64, :], ut[0:64, :], bc_sb[0:64, :], ALU.mult
                        )
                        # partition shift 0-63 -> 64-127 via SBUF-SBUF DMA
                        nc.sync.dma_start(ctx_sb[64:128, np_, :], ctmp[0:64, :])

            def prefetch_pair(pp_):
                vt = vatp.tile([P, NB, SQ, 130], FP32R, tag="vat", name=f"vt_{pp_}")
                for b in range(NB):
                    nc.sync.dma_start(vt[:, b], vf_blk[:, b, :, pp_, :])
                ktts = []
                for sb4 in range(NB):
                    kq = kttp.tile(
                        [P, SL], FP32R, tag="ktt", name=f"ktt_{pp_}_{sb4}"
                    )
                    nc.sync.dma_start(kq[:], kt_dr[:, pp_, ds(sb4 * SL, SL)])
                    ktts.append(kq)
                return vt, ktts

            tiles = {0: prefetch_pair(0)}
            norm_pend = None
            for p in range(PAIRS):
                utA = psAcc.tile([P, SL], FP32, tag="accA")
                utB = psAcc.tile([P, SL], FP32, tag="accB")
                vt, ktts = tiles.pop(p)
                # software pipeline: issue S^T/exp for chunk c+1 before the
                # U^T matmuls of chunk c, so the in-order PE never stalls on
                # ACT; the previous pair's normalize is likewise deferred into
                # this pair's stream so its PE broadcast never waits on DVE.
                pend = None
                for c in range(CH):
                    ktt = ktts[c // SQ][:, ts(c % SQ, P)]
                    ets = []
                    for j in range(2):
                        st = psA.tile([P, SL], FP32, tag="mm")
                        nc.tensor.matmul(
                            st[:],
                            ktt[ds(j * 64, 64), :],
                            qt_sb[ds(j * 64, 64), p, :],
                            start=True,
                            stop=True,
                        )
                        et = etp.tile([P, SL], FP32R, tag="et")
                        nc.scalar.activation(
                            out=et[:], in_=st[:], func=AF.Exp, scale=0.125
                        )
                        ets.append(et)
                    if c == 7 and norm_pend is not None:
                        emit_normalize(*norm_pend)
                        norm_pend = None
                    if c == 4 and p + 1 < PAIRS:
                        tiles[p + 1] = prefetch_pair(p + 1)
                    if pend is not None:
                        pc, pets, pv = pend
                        for j, ut in enumerate((utA, utB)):
                            nc.tensor.matmul(
                                ut[:65, :],
                                pv[:, ds(j * 65, 65)],
                                pets[j][:],
                                start=(pc == 0),
                                stop=False,
                            )
                    pend = (c, ets, vt[:, c // SQ, c % SQ, :])
                pc, pets, pv = pend
                for j, ut in enumerate((utA, utB)):
                    nc.tensor.matmul(
                        ut[:65, :],
                        pv[:, ds(j * 65, 65)],
                        pets[j][:],
                        start=False,
                        stop=True,
                    )
                norm_pend = (p, utA, utB)
            emit_normalize(*norm_pend)

            # ---- output projection + residual + LayerNorm ----
            out_sb = big.tile([P, SQ, D], FP32, tag="out")
            wo_sb = wpool.tile([P, KC, D], FP32R, tag="wrhs")
            for k in range(KC):
                nc.sync.dma_start(wo_sb[:, k, :], wo_d[ts(k, P), :])
            for n in range(2):
                for i in range(SQ):
                    pp = psA.tile([P, 512], FP32, tag="mm")
                    for p in range(PAIRS):
                        nc.tensor.matmul(
                            pp[:],
                            ctx_sb[:, p, ts(i, P)],
                            wo_sb[:, p, ds(n * 512, 512)],
                            start=(p == 0),
                            stop=(p == PAIRS - 1),
                        )
                    res = stream.tile([P, 512], FP32R, tag="res")
                    nc.sync.dma_start(res[:], xq_d[ts(i, P), ds(n * 512, 512)])
                    tmp = stream.tile([P, 512], FP32, tag="otmp")
                    nc.vector.tensor_tensor(tmp[:], pp[:], res[:], ALU.add)
                    nc.vector.tensor_tensor(
                        out_sb[:, i, ds(n * 512, 512)],
                        tmp[:],
                        bo_b[:, ds(n * 512, 512)],
                        ALU.add,
                    )

            for i in range(SQ):
                row = out_sb[:, i, :]
                stats = small.tile([P, 2, 6], FP32, tag="stats")
                nc.vector.bn_stats(stats[:, 0, :], row[:, 0:512])
                nc.vector.bn_stats(stats[:, 1, :], row[:, 512:1024])
                mv = small.tile([P, 2], FP32, tag="mv")
                nc.vector.bn_aggr(mv[:], stats[:])
                std = small.tile([P, 1], FP32, tag="std")
                nc.scalar.activation(
                    out=std[:], in_=mv[:, 1:2], func=AF.Sqrt, bias=eps_t[:], scale=1.0
                )
                rstd = small.tile([P, 1], FP32, tag="rstd")
                nc.vector.reciprocal(out=rstd[:], in_=std[:])
                ytile = wide.tile([P, D], FP32, tag="y")
                nc.vector.tensor_scalar(
                    out=ytile[:],
                    in0=row,
                    scalar1=mv[:, 0:1],
                    scalar2=rstd[:],
                    op0=ALU.subtract,
                    op1=ALU.mult,
                )
                nc.vector.tensor_tensor(ytile[:], ytile[:], gam_b[:], ALU.mult)
                nc.vector.tensor_tensor(ytile[:], ytile[:], bet_b[:], ALU.add)
                nc.sync.dma_start(y_d[ts(i, P), :], ytile[:])

    nc.compile()
    return nc


def get_nc():
    if "nc" not in _NC_CACHE:
        _NC_CACHE["nc"] = build_nc()
    return _NC_CACHE["nc"]


def kernel(
    query,
    key,
    value,
    Wq,
    bq,
    Wk,
    bk,
    Wv,
    bv,
    Wo,
    bo,
    ln_gamma,
    ln_beta,
    _trace=False,
    _trace_cores=None,
):
    query = np.ascontiguousarray(np.asarray(query, dtype=np.float32))
    key = np.ascontiguousarray(np.asarray(key, dtype=np.float32))
    value = np.ascontiguousarray(np.asarray(value, dtype=np.float32))
    shared = {
        "wq": np.ascontiguousarray(np.asarray(Wq, np.float32)),
        "wk": np.ascontiguousarray(np.asarray(Wk, np.float32)),
        "wv": np.ascontiguousarray(np.asarray(Wv, np.float32)),
        "wo": np.ascontiguousarray(np.asarray(Wo, np.float32)),
        "bq": np.ascontiguousarray(np.asarray(bq, np.float32)),
        "bk": np.ascontiguousarray(np.asarray(bk, np.float32)),
        "bv": np.ascontiguousarray(np.asarray(bv, np.float32)),
        "bo": np.ascontiguousarray(np.asarray(bo, np.float32)),
        "gam": np.ascontiguousarray(np.asarray(ln_gamma, np.float32)),
        "bet": np.ascontiguousarray(np.asarray(ln_beta, np.float32)),
        "ident": np.eye(P, dtype=np.float32),
        "ones": np.ones((P, 64), dtype=np.float32),
    }
    in_maps = []
    for c in range(N_CORES):
        b, r = divmod(c, NB)
        rows = slice(r * SL, (r + 1) * SL)
        m = dict(shared)
        m["xq"] = np.ascontiguousarray(query[b, rows, :])
        m["xk"] = np.ascontiguousarray(key[b])
        m["xv"] = np.ascontiguousarray(value[b])
        in_maps.append(m)

    nc = get_nc()
    res = run_bass_kernel_spmd(
        nc,
        in_maps,
        list(range(N_CORES)),
        trace=_trace,
        trace_cores=_trace_cores,
    )
    out = np.empty((B, S, D), dtype=np.float32)
    for c in range(N_CORES):
        b, r = divmod(c, NB)
        out[b, r * SL : (r + 1) * SL, :] = res.results[c]["y"]
    if _trace:
        return out, res
    return out


# revision 23
# speedup vs baseline: 1.0654x; 1.0654x over previous
"""Multi-head attention + residual + LayerNorm on 8 Trainium2 NeuronCores.

Sharding: core c in 0..7 handles batch b = c//4 and query-row quarter
r = c%4 (rows 512r..512r+512 of S=2048), with ALL 16 heads.  key/value
are replicated to every core (host-side staging); each core computes the
full-sequence K^T and V projections itself — measured collectives on this
stack cost ~130us per 2MB AllGather, far more than the ~70us of redundant
PE work, and the local pipeline keeps the PE clock warm.

Per core:
  - transpose x tiles on PE (fp32r, via identity), project:
      Q^T [1024, 512] (own rows),  K^T [1024, 2048] -> local DRAM,
      V [2048, 8, 130] pair-blocks with ones columns -> local DRAM
  - per head pair p, per sk chunk c: S^T = K_h Q_h^T  (PSUM) -> exp
    (ACT, scale 1/8) -> U^T accumulation with lhsT = V_aug; the ones
    column makes row 64 the softmax denominators
  - normalize: reciprocal of sums row, PE K=1 broadcast to 64
    partitions, multiply -> ctx^T [128, 8, 512]
  - out = ctx @ Wo + bo + residual -> LayerNorm -> y rows [512, 1024]

All matmuls in float32r (full-rate fp32 PE path, ~2e-4 rel err).
"""

import sys

if "/opt/trn_rl_repo" not in sys.path:
    sys.path.insert(0, "/opt/trn_rl_repo")

import numpy as np

import concourse.bacc as bacc
import concourse.bass as bass
import concourse.mybir as mybir
import concourse.tile as tile
from concourse.bass import ds, ts
from concourse.bass_utils import run_bass_kernel_spmd

FP32R = mybir.dt.float32r
FP32 = mybir.dt.float32
AF = mybir.ActivationFunctionType
ALU = mybir.AluOpType

N_CORES = 8
B = 2
S = 2048
D = 1024
H = 16
DK = 64
P = 128

SL = S // 4  # 512 local query rows per core
KC = D // P  # 8 contraction chunks over d_model
SQ = SL // P  # 4 sq subchunks of 128 (per 512-row block)
CH = S // P  # 16 sk chunks
PAIRS = H // 2  # 8 head pairs
NB = 4  # 512-row blocks of the full sequence
EPS = 1e-5

_NC_CACHE = {}


def build_nc():
    nc = bacc.Bacc(num_devices=N_CORES)

    xq_d = nc.dram_tensor("xq", [SL, D], FP32R, kind="ExternalInput")
    xk_d = nc.dram_tensor("xk", [S, D], FP32R, kind="ExternalInput")
    xv_d = nc.dram_tensor("xv", [S, D], FP32R, kind="ExternalInput")
    wq_d = nc.dram_tensor("wq", [D, D], FP32R, kind="ExternalInput")
    wk_d = nc.dram_tensor("wk", [D, D], FP32R, kind="ExternalInput")
    wv_d = nc.dram_tensor("wv", [D, D], FP32R, kind="ExternalInput")
    wo_d = nc.dram_tensor("wo", [D, D], FP32R, kind="ExternalInput")
    bq_d = nc.dram_tensor("bq", [D], FP32, kind="ExternalInput")
    bk_d = nc.dram_tensor("bk", [D], FP32, kind="ExternalInput")
    bv_d = nc.dram_tensor("bv", [D], FP32, kind="ExternalInput")
    bo_d = nc.dram_tensor("bo", [D], FP32, kind="ExternalInput")
    gam_d = nc.dram_tensor("gam", [D], FP32, kind="ExternalInput")
    bet_d = nc.dram_tensor("bet", [D], FP32, kind="ExternalInput")
    ident_d = nc.dram_tensor("ident", [P, P], FP32R, kind="ExternalInput")
    ones_d = nc.dram_tensor("ones", [P, 64], FP32R, kind="ExternalInput")

    y_d = nc.dram_tensor("y", [SL, D], FP32, kind="ExternalOutput")

    # local DRAM for the full-sequence K^T and augmented V
    kt_d = nc.dram_tensor("ktf", [D, S], FP32R)
    vf_d = nc.dram_tensor("vf", [S, PAIRS, 130], FP32R)

    with tile.TileContext(nc) as tc:
        with (
            tc.tile_pool(name="consts", bufs=1) as consts,
            tc.tile_pool(name="big", bufs=1) as big,
            tc.tile_pool(name="xtp", bufs=1) as xtp,
            tc.tile_pool(name="wide", bufs=1) as wide,
            tc.tile_pool(name="xnp", bufs=4) as xnp,
            tc.tile_pool(name="wpool", bufs=1) as wpool,
            tc.tile_pool(name="stream", bufs=3) as stream,
            tc.tile_pool(name="kttp", bufs=7) as kttp,
            tc.tile_pool(name="etp", bufs=4) as etp,
            tc.tile_pool(name="vat", bufs=2) as vatp,
            tc.tile_pool(name="small", bufs=2) as small,
            tc.tile_pool(name="psA", bufs=3, space="PSUM") as psA,
            tc.tile_pool(name="psAcc", bufs=2, space="PSUM") as psAcc,
            tc.tile_pool(name="psB", bufs=1, space="PSUM") as psB,
        ):
            # ---- constants ----
            ident = consts.tile([P, P], FP32R)
            nc.sync.dma_start(ident[:], ident_d[:])
            ones64 = consts.tile([P, 64], FP32R)
            nc.sync.dma_start(ones64[:], ones_d[:])
            bq_sb = consts.tile([P, KC], FP32)
            nc.sync.dma_start(bq_sb[:], bq_d.rearrange("(m q) -> q m", q=P))
            bk_sb = consts.tile([P, KC], FP32)
            nc.sync.dma_start(bk_sb[:], bk_d.rearrange("(m q) -> q m", q=P))

            def bcast_load(src, tag):
                t = consts.tile([P, D], FP32, tag=tag)
                ap = bass.AP(tensor=src, offset=0, ap=[[0, P], [1, D]])
                nc.gpsimd.dma_start(out=t[:], in_=ap)
                return t

            bv_b = bcast_load(bv_d, "bv_b")
            bo_b = bcast_load(bo_d, "bo_b")
            gam_b = bcast_load(gam_d, "gam_b")
            bet_b = bcast_load(bet_d, "bet_b")
            eps_t = consts.tile([P, 1], FP32)
            nc.vector.memset(eps_t[:], EPS)

            def load_xT(x_d, row0):
                """x rows [row0:row0+512] -> x^T SBUF [128, KC, 512]."""
                xT = xtp.tile([P, KC, SL], FP32R, tag="xT")
                for i in range(SQ):
                    xt = xnp.tile([P, D], FP32R, tag="xnat")
                    nc.sync.dma_start(xt[:], x_d[ds(row0 + i * P, P), :])
                    for j in range(KC):
                        pt = psA.tile([P, P], FP32R, tag="mm")
                        nc.tensor.transpose(pt[:], xt[:, ts(j, P)], ident[:])
                        nc.vector.tensor_copy(xT[:, j, ts(i, P)], pt[:])
                return xT

            # ---- K^T full sequence -> kt_d, block by block ----
            kt_dr = kt_d.rearrange("(m q) s -> q m s", q=P)
            wk_sb = wpool.tile([P, KC, D], FP32R, tag="wrhs")
            for k in range(KC):
                nc.sync.dma_start(wk_sb[:, k, :], wk_d[ts(k, P), :])
            for blk in range(NB):
                xkT = load_xT(xk_d, blk * SL)
                for m in range(KC):
                    pp = psA.tile([P, SL], FP32, tag="mm")
                    for k in range(KC):
                        nc.tensor.matmul(
                            pp[:],
                            wk_sb[:, k, ts(m, P)],
                            xkT[:, k, :],
                            start=(k == 0),
                            stop=(k == KC - 1),
                        )
                    kev = stream.tile([P, SL], FP32R, tag="kev")
                    nc.scalar.activation(
                        out=kev[:],
                        in_=pp[:],
                        func=AF.Identity,
                        bias=bk_sb[:, m : m + 1],
                    )
                    nc.sync.dma_start(kt_dr[:, m, ds(blk * SL, SL)], kev[:])

            # ---- V full sequence -> vf_d (pair-augmented layout) ----
            vf_dr = vf_d.rearrange("(i q) p e -> q i p e", q=P)
            wv_sb = wpool.tile([P, KC, D], FP32R, tag="wrhs")
            for k in range(KC):
                nc.sync.dma_start(wv_sb[:, k, :], wv_d[ts(k, P), :])
            for blk in range(NB):
                xvT = load_xT(xv_d, blk * SL)
                for n in range(2):
                    for i in range(SQ):
                        pp = psA.tile([P, 512], FP32, tag="mm")
                        for k in range(KC):
                            nc.tensor.matmul(
                                pp[:],
                                xvT[:, k, ts(i, P)],
                                wv_sb[:, k, ds(n * 512, 512)],
                                start=(k == 0),
                                stop=(k == KC - 1),
                            )
                        # vtmp holds [4 pairs x (V_even |1| V_odd |1)] = 520 cols
                        vtmp = stream.tile([P, 4, 130], FP32R, tag="vtmp")
                        vdst = vtmp[:].rearrange("q pl (j e) -> q pl j e", e=65)
                        nc.vector.tensor_tensor(
                            vdst[:, :, :, 0:64],
                            pp[:].rearrange("q (pl j e) -> q pl j e", pl=4, j=2),
                            bv_b[:, ds(n * 512, 512)].rearrange(
                                "q (pl j e) -> q pl j e", pl=4, j=2
                            ),
                            ALU.add,
                        )
                        nc.vector.tensor_copy(
                            vdst[:, :, :, 64:65], ones64[:, 0:8, None].rearrange(
                                "q (pl j) o -> q pl j o", pl=4
                            )
                        )
                        ii = blk * SQ + i
                        nc.sync.dma_start(vf_dr[:, ii, ds(n * 4, 4), :], vtmp[:])

            # ---- Q^T (own rows) ----
            xqT = load_xT(xq_d, 0)
            qt_sb = big.tile([P, KC, SL], FP32R, tag="qt")
            wq_sb = wpool.tile([P, KC, D], FP32R, tag="wrhs")
            for k in range(KC):
                nc.sync.dma_start(wq_sb[:, k, :], wq_d[ts(k, P), :])
            for m in range(KC):
                pp = psA.tile([P, SL], FP32, tag="mm")
                for k in range(KC):
                    nc.tensor.matmul(
                        pp[:],
                        wq_sb[:, k, ts(m, P)],
                        xqT[:, k, :],
                        start=(k == 0),
                        stop=(k == KC - 1),
                    )
                nc.scalar.activation(
                    out=qt_sb[:, m, :],
                    in_=pp[:],
                    func=AF.Identity,
                    bias=bq_sb[:, m : m + 1],
                )

            # ---- attention ----
            ctx_sb = big.tile([P, PAIRS, SL], FP32R, tag="ctx")
            vf_blk = vf_d.rearrange("(b i q) p e -> q b i p e", b=NB, q=P)

            def emit_normalize(np_, uA, uB):
                # rows 0..63 of ut / row 64 -> ctx_sb[:, np_, :]
                for j, ut in enumerate((uA, uB)):
                    rec = small.tile([P, SL], FP32R, tag="rec")
                    with nc.allow_low_precision(
                        reason="float32r is bit-identical to float32 in SBUF"
                    ):
                        nc.vector.reciprocal(out=rec[64:65, :], in_=ut[64:65, :])
                    bc = psB.tile([P, SL], FP32, tag="bc")
                    nc.tensor.matmul(
                        bc[0:64, :],
                        ones64[64:65, :],
                        rec[64:65, :],
                        start=True,
                        stop=True,
                    )
                    bc_sb = small.tile([P, SL], FP32, tag="bcs")
                    nc.vector.tensor_copy(bc_sb[0:64, :], bc[0:64, :])
                    if j == 0:
                        nc.vector.tensor_tensor(
                            ctx_sb[0:64, np_, :], ut[0:64, :], bc_sb[0:64, :], ALU.mult
                        )
                    else:
                        ctmp = small.tile([P, SL], FP32R, tag="ctmp")
                        nc.vector.tensor_tensor(
                            ctmp[0:64, :], ut[0:64, :], bc_sb[0:64, :], ALU.mult
                        )
                        # partition shift 0-63 -> 64-127 via SBUF-SBUF DMA
                        nc.sync.dma_start(ctx_sb[64:128, np_, :], ctmp[0:64, :])

            def prefetch_pair(pp_):
                vt = vatp.tile([P, NB, SQ, 130], FP32R, tag="vat", name=f"vt_{pp_}")
                for b in range(NB):
                    nc.sync.dma_start(vt[:, b], vf_blk[:, b, :, pp_, :])
                ktts = []
                for sb4 in range(NB):
                    kq = kttp.tile(
                        [P, SL], FP32R, tag="ktt", name=f"ktt_{pp_}_{sb4}"
                    )
                    nc.sync.dma_start(kq[:], kt_dr[:, pp_, ds(sb4 * SL, SL)])
                    ktts.append(kq)
                return vt, ktts

            tiles = {0: prefetch_pair(0)}
            norm_pend = None
            for p in range(PAIRS):
                utA = psAcc.tile([P, SL], FP32, tag="accA")
                utB = psAcc.tile([P, SL], FP32, tag="accB")
                vt, ktts = tiles.pop(p)
                # software pipeline: issue S^T/exp for chunk c+1 before the
                # U^T matmuls of chunk c, so the in-order PE never stalls on
                # ACT; the previous pair's normalize is likewise deferred into
                # this pair's stream so its PE broadcast never waits on DVE.
                pend = None
                for c in range(CH):
                    ktt = ktts[c // SQ][:, ts(c % SQ, P)]
                    ets = []
                    for j in range(2):
                        st = psA.tile([P, SL], FP32, tag="mm")
                        nc.tensor.matmul(
                            st[:],
                            ktt[ds(j * 64, 64), :],
                            qt_sb[ds(j * 64, 64), p, :],
                            start=True,
                            stop=True,
                        )
                        et = etp.tile([P, SL], FP32R, tag="et")
                        nc.scalar.activation(
                            out=et[:], in_=st[:], func=AF.Exp, scale=0.125
                        )
                        ets.append(et)
                    if c == 7 and norm_pend is not None:
                        emit_normalize(*norm_pend)
                        norm_pend = None
                    if c == 4 and p + 1 < PAIRS:
                        tiles[p + 1] = prefetch_pair(p + 1)
                    if pend is not None:
                        pc, pets, pv = pend
                        for j, ut in enumerate((utA, utB)):
                            nc.tensor.matmul(
                                ut[:65, :],
                                pv[:, ds(j * 65, 65)],
                                pets[j][:],
                                start=(pc == 0),
                                stop=False,
                            )
                    pend = (c, ets, vt[:, c // SQ, c % SQ, :])
                pc, pets, pv = pend
                for j, ut in enumerate((utA, utB)):
                    nc.tensor.matmul(
                        ut[:65, :],
                        pv[:, ds(j * 65, 65)],
                        pets[j][:],
                        start=False,
                        stop=True,
                    )
                norm_pend = (p, utA, utB)
            emit_normalize(*norm_pend)

            # ---- output projection + residual + LayerNorm ----
            out_sb = big.tile([P, SQ, D], FP32, tag="out")
            wo_sb = wpool.tile([P, KC, D], FP32R, tag="wrhs")
            for k in range(KC):
                nc.sync.dma_start(wo_sb[:, k, :], wo_d[ts(k, P), :])
            for n in range(2):
                for i in range(SQ):
                    pp = psA.tile([P, 512], FP32, tag="mm")
                    for p in range(PAIRS):
                        nc.tensor.matmul(
                            pp[:],
                            ctx_sb[:, p, ts(i, P)],
                            wo_sb[:, p, ds(n * 512, 512)],
                            start=(p == 0),
                            stop=(p == PAIRS - 1),
                        )
                    res = stream.tile([P, 512], FP32R, tag="res")
                    nc.sync.dma_start(res[:], xq_d[ts(i, P), ds(n * 512, 512)])
                    tmp = stream.tile([P, 512], FP32, tag="otmp")
                    nc.vector.tensor_tensor(tmp[:], pp[:], res[:], ALU.add)
                    nc.vector.tensor_tensor(
                        out_sb[:, i, ds(n * 512, 512)],
                        tmp[:],
                        bo_b[:, ds(n * 512, 512)],
                        ALU.add,
                    )

            for i in range(SQ):
                row = out_sb[:, i, :]
                stats = small.tile([P, 2, 6], FP32, tag="stats")
                nc.vector.bn_stats(stats[:, 0, :], row[:, 0:512])
                nc.vector.bn_stats(stats[:, 1, :], row[:, 512:1024])
                mv = small.tile([P, 2], FP32, tag="mv")
                nc.vector.bn_aggr(mv[:], stats[:])
                std = small.tile([P, 1], FP32, tag="std")
                nc.scalar.activation(
                    out=std[:], in_=mv[:, 1:2], func=AF.Sqrt, bias=eps_t[:], scale=1.0
                )
                rstd = small.tile([P, 1], FP32, tag="rstd")
                nc.vector.reciprocal(out=rstd[:], in_=std[:])
                ytile = wide.tile([P, D], FP32, tag="y")
                nc.vector.tensor_scalar(
                    out=ytile[:],
                    in0=row,
                    scalar1=mv[:, 0:1],
                    scalar2=rstd[:],
                    op0=ALU.subtract,
                    op1=ALU.mult,
                )
                nc.vector.tensor_tensor(ytile[:], ytile[:], gam_b[:], ALU.mult)
                nc.vector.tensor_tensor(ytile[:], ytile[:], bet_b[:], ALU.add)
                nc.sync.dma_start(y_d[ts(i, P), :], ytile[:])

    nc.compile()
    return nc


def get_nc():
    if "nc" not in _NC_CACHE:
        _NC_CACHE["nc"] = build_nc()
    return _NC_CACHE["nc"]


def kernel(
    query,
    key,
    value,
    Wq,
    bq,
    Wk,
    bk,
    Wv,
    bv,
    Wo,
    bo,
    ln_gamma,
    ln_beta,
    _trace=False,
    _trace_cores=None,
):
    query = np.ascontiguousarray(np.asarray(query, dtype=np.float32))
    key = np.ascontiguousarray(np.asarray(key, dtype=np.float32))
    value = np.ascontiguousarray(np.asarray(value, dtype=np.float32))
    shared = {
        "wq": np.ascontiguousarray(np.asarray(Wq, np.float32)),
        "wk": np.ascontiguousarray(np.asarray(Wk, np.float32)),
        "wv": np.ascontiguousarray(np.asarray(Wv, np.float32)),
        "wo": np.ascontiguousarray(np.asarray(Wo, np.float32)),
        "bq": np.ascontiguousarray(np.asarray(bq, np.float32)),
        "bk": np.ascontiguousarray(np.asarray(bk, np.float32)),
        "bv": np.ascontiguousarray(np.asarray(bv, np.float32)),
        "bo": np.ascontiguousarray(np.asarray(bo, np.float32)),
        "gam": np.ascontiguousarray(np.asarray(ln_gamma, np.float32)),
        "bet": np.ascontiguousarray(np.asarray(ln_beta, np.float32)),
        "ident": np.eye(P, dtype=np.float32),
        "ones": np.ones((P, 64), dtype=np.float32),
    }
    in_maps = []
    for c in range(N_CORES):
        b, r = divmod(c, NB)
        rows = slice(r * SL, (r + 1) * SL)
        m = dict(shared)
        m["xq"] = np.ascontiguousarray(query[b, rows, :])
        m["xk"] = np.ascontiguousarray(key[b])
        m["xv"] = np.ascontiguousarray(value[b])
        in_maps.append(m)

    nc = get_nc()
    res = run_bass_kernel_spmd(
        nc,
        in_maps,
        list(range(N_CORES)),
        trace=_trace,
        trace_cores=_trace_cores,
    )
    out = np.empty((B, S, D), dtype=np.float32)
    for c in range(N_CORES):
        b, r = divmod(c, NB)
        out[b, r * SL : (r + 1) * SL, :] = res.results[c]["y"]
    if _trace:
        return out, res
    return out


# revision 24
# speedup vs baseline: 1.0768x; 1.0107x over previous
"""Multi-head attention + residual + LayerNorm on 8 Trainium2 NeuronCores.

Sharding: core c in 0..7 handles batch b = c//4 and query-row quarter
r = c%4 (rows 512r..512r+512 of S=2048), with ALL 16 heads.  key/value
are replicated to every core (host-side staging); each core computes the
full-sequence K^T and V projections itself — measured collectives on this
stack cost ~130us per 2MB AllGather, far more than the ~70us of redundant
PE work, and the local pipeline keeps the PE clock warm.

Per core:
  - transpose x tiles on PE (fp32r, via identity), project:
      Q^T [1024, 512] (own rows),  K^T [1024, 2048] -> local DRAM,
      V [2048, 8, 130] pair-blocks with ones columns -> local DRAM
  - per head pair p, per sk chunk c: S^T = K_h Q_h^T  (PSUM) -> exp
    (ACT, scale 1/8) -> U^T accumulation with lhsT = V_aug; the ones
    column makes row 64 the softmax denominators
  - normalize: reciprocal of sums row, PE K=1 broadcast to 64
    partitions, multiply -> ctx^T [128, 8, 512]
  - out = ctx @ Wo + bo + residual -> LayerNorm -> y rows [512, 1024]

All matmuls in float32r (full-rate fp32 PE path, ~2e-4 rel err).
"""

import sys

if "/opt/trn_rl_repo" not in sys.path:
    sys.path.insert(0, "/opt/trn_rl_repo")

import numpy as np

import concourse.bacc as bacc
import concourse.bass as bass
import concourse.mybir as mybir
import concourse.tile as tile
from concourse.bass import ds, ts
from concourse.bass_utils import run_bass_kernel_spmd

FP32R = mybir.dt.float32r
FP32 = mybir.dt.float32
AF = mybir.ActivationFunctionType
ALU = mybir.AluOpType

N_CORES = 8
B = 2
S = 2048
D = 1024
H = 16
DK = 64
P = 128

SL = S // 4  # 512 local query rows per core
KC = D // P  # 8 contraction chunks over d_model
SQ = SL // P  # 4 sq subchunks of 128 (per 512-row block)
CH = S // P  # 16 sk chunks
PAIRS = H // 2  # 8 head pairs
NB = 4  # 512-row blocks of the full sequence
EPS = 1e-5

_NC_CACHE = {}


def build_nc():
    nc = bacc.Bacc(num_devices=N_CORES)

    xq_d = nc.dram_tensor("xq", [SL, D], FP32R, kind="ExternalInput")
    xk_d = nc.dram_tensor("xk", [S, D], FP32R, kind="ExternalInput")
    xv_d = nc.dram_tensor("xv", [S, D], FP32R, kind="ExternalInput")
    wq_d = nc.dram_tensor("wq", [D, D], FP32R, kind="ExternalInput")
    wk_d = nc.dram_tensor("wk", [D, D], FP32R, kind="ExternalInput")
    wv_d = nc.dram_tensor("wv", [D, D], FP32R, kind="ExternalInput")
    wo_d = nc.dram_tensor("wo", [D, D], FP32R, kind="ExternalInput")
    bq_d = nc.dram_tensor("bq", [D], FP32, kind="ExternalInput")
    bk_d = nc.dram_tensor("bk", [D], FP32, kind="ExternalInput")
    bv_d = nc.dram_tensor("bv", [D], FP32, kind="ExternalInput")
    bo_d = nc.dram_tensor("bo", [D], FP32, kind="ExternalInput")
    gam_d = nc.dram_tensor("gam", [D], FP32, kind="ExternalInput")
    bet_d = nc.dram_tensor("bet", [D], FP32, kind="ExternalInput")
    ident_d = nc.dram_tensor("ident", [P, P], FP32R, kind="ExternalInput")
    ones_d = nc.dram_tensor("ones", [P, 64], FP32R, kind="ExternalInput")

    y_d = nc.dram_tensor("y", [SL, D], FP32, kind="ExternalOutput")

    # local DRAM for the full-sequence K^T and augmented V
    kt_d = nc.dram_tensor("ktf", [D, S], FP32R)
    vf_d = nc.dram_tensor("vf", [S, PAIRS, 130], FP32R)

    with tile.TileContext(nc) as tc:
        with (
            tc.tile_pool(name="consts", bufs=1) as consts,
            tc.tile_pool(name="big", bufs=1) as big,
            tc.tile_pool(name="xtp", bufs=1) as xtp,
            tc.tile_pool(name="wide", bufs=1) as wide,
            tc.tile_pool(name="xnp", bufs=4) as xnp,
            tc.tile_pool(name="wpool", bufs=1) as wpool,
            tc.tile_pool(name="stream", bufs=3) as stream,
            tc.tile_pool(name="kttp", bufs=7) as kttp,
            tc.tile_pool(name="etp", bufs=4) as etp,
            tc.tile_pool(name="vat", bufs=2) as vatp,
            tc.tile_pool(name="small", bufs=2) as small,
            tc.tile_pool(name="psA", bufs=3, space="PSUM") as psA,
            tc.tile_pool(name="psAcc", bufs=2, space="PSUM") as psAcc,
            tc.tile_pool(name="psB", bufs=1, space="PSUM") as psB,
        ):
            # ---- constants ----
            ident = consts.tile([P, P], FP32R)
            nc.sync.dma_start(ident[:], ident_d[:])
            ones64 = consts.tile([P, 64], FP32R)
            nc.sync.dma_start(ones64[:], ones_d[:])
            bq_sb = consts.tile([P, KC], FP32)
            nc.sync.dma_start(bq_sb[:], bq_d.rearrange("(m q) -> q m", q=P))
            bk_sb = consts.tile([P, KC], FP32)
            nc.sync.dma_start(bk_sb[:], bk_d.rearrange("(m q) -> q m", q=P))

            def bcast_load(src, tag):
                t = consts.tile([P, D], FP32, tag=tag)
                ap = bass.AP(tensor=src, offset=0, ap=[[0, P], [1, D]])
                nc.gpsimd.dma_start(out=t[:], in_=ap)
                return t

            bv_b = bcast_load(bv_d, "bv_b")
            bo_b = bcast_load(bo_d, "bo_b")
            gam_b = bcast_load(gam_d, "gam_b")
            bet_b = bcast_load(bet_d, "bet_b")
            eps_t = consts.tile([P, 1], FP32)
            nc.vector.memset(eps_t[:], EPS)

            def load_xT(x_d, row0):
                """x rows [row0:row0+512] -> x^T SBUF [128, KC, 512]."""
                xT = xtp.tile([P, KC, SL], FP32R, tag="xT")
                for i in range(SQ):
                    xt = xnp.tile([P, D], FP32R, tag="xnat")
                    nc.sync.dma_start(xt[:], x_d[ds(row0 + i * P, P), :])
                    for j in range(KC):
                        pt = psA.tile([P, P], FP32R, tag="mm")
                        nc.tensor.transpose(pt[:], xt[:, ts(j, P)], ident[:])
                        nc.vector.tensor_copy(xT[:, j, ts(i, P)], pt[:])
                return xT

            # ---- K^T full sequence -> kt_d, block by block ----
            kt_dr = kt_d.rearrange("(m q) s -> q m s", q=P)
            wk_sb = wpool.tile([P, KC, D], FP32R, tag="wrhs")
            for k in range(KC):
                nc.sync.dma_start(wk_sb[:, k, :], wk_d[ts(k, P), :])
            for blk in range(NB):
                xkT = load_xT(xk_d, blk * SL)
                for m in range(KC):
                    pp = psA.tile([P, SL], FP32, tag="mm")
                    for k in range(KC):
                        nc.tensor.matmul(
                            pp[:],
                            wk_sb[:, k, ts(m, P)],
                            xkT[:, k, :],
                            start=(k == 0),
                            stop=(k == KC - 1),
                        )
                    kev = stream.tile([P, SL], FP32R, tag="kev")
                    nc.scalar.activation(
                        out=kev[:],
                        in_=pp[:],
                        func=AF.Identity,
                        bias=bk_sb[:, m : m + 1],
                    )
                    nc.sync.dma_start(kt_dr[:, m, ds(blk * SL, SL)], kev[:])

            # ---- V full sequence -> vf_d (pair-augmented layout) ----
            vf_dr = vf_d.rearrange("(i q) p e -> q i p e", q=P)
            wv_sb = wpool.tile([P, KC, D], FP32R, tag="wrhs")
            for k in range(KC):
                nc.sync.dma_start(wv_sb[:, k, :], wv_d[ts(k, P), :])
            for blk in range(NB):
                xvT = load_xT(xv_d, blk * SL)
                for n in range(2):
                    for i in range(SQ):
                        pp = psA.tile([P, 512], FP32, tag="mm")
                        for k in range(KC):
                            nc.tensor.matmul(
                                pp[:],
                                xvT[:, k, ts(i, P)],
                                wv_sb[:, k, ds(n * 512, 512)],
                                start=(k == 0),
                                stop=(k == KC - 1),
                            )
                        # vtmp holds [4 pairs x (V_even |1| V_odd |1)] = 520 cols
                        vtmp = stream.tile([P, 4, 130], FP32R, tag="vtmp")
                        vdst = vtmp[:].rearrange("q pl (j e) -> q pl j e", e=65)
                        nc.vector.tensor_tensor(
                            vdst[:, :, :, 0:64],
                            pp[:].rearrange("q (pl j e) -> q pl j e", pl=4, j=2),
                            bv_b[:, ds(n * 512, 512)].rearrange(
                                "q (pl j e) -> q pl j e", pl=4, j=2
                            ),
                            ALU.add,
                        )
                        nc.vector.tensor_copy(
                            vdst[:, :, :, 64:65], ones64[:, 0:8, None].rearrange(
                                "q (pl j) o -> q pl j o", pl=4
                            )
                        )
                        ii = blk * SQ + i
                        nc.sync.dma_start(vf_dr[:, ii, ds(n * 4, 4), :], vtmp[:])

            # ---- Q^T (own rows) ----
            xqT = load_xT(xq_d, 0)
            qt_sb = big.tile([P, KC, SL], FP32R, tag="qt")
            wq_sb = wpool.tile([P, KC, D], FP32R, tag="wrhs")
            for k in range(KC):
                nc.sync.dma_start(wq_sb[:, k, :], wq_d[ts(k, P), :])
            for m in range(KC):
                pp = psA.tile([P, SL], FP32, tag="mm")
                for k in range(KC):
                    nc.tensor.matmul(
                        pp[:],
                        wq_sb[:, k, ts(m, P)],
                        xqT[:, k, :],
                        start=(k == 0),
                        stop=(k == KC - 1),
                    )
                nc.scalar.activation(
                    out=qt_sb[:, m, :],
                    in_=pp[:],
                    func=AF.Identity,
                    bias=bq_sb[:, m : m + 1],
                )

            # ---- attention ----
            ctx_sb = big.tile([P, PAIRS, SL], FP32R, tag="ctx")
            vf_blk = vf_d.rearrange("(b i q) p e -> q b i p e", b=NB, q=P)

            def emit_normalize(np_, uA, uB):
                # rows 0..63 of ut / row 64 -> ctx_sb[:, np_, :]
                for j, ut in enumerate((uA, uB)):
                    rec = small.tile([P, SL], FP32R, tag="rec")
                    with nc.allow_low_precision(
                        reason="float32r is bit-identical to float32 in SBUF"
                    ):
                        nc.vector.reciprocal(out=rec[64:65, :], in_=ut[64:65, :])
                    bc = psB.tile([P, SL], FP32, tag="bc")
                    nc.tensor.matmul(
                        bc[0:64, :],
                        ones64[64:65, :],
                        rec[64:65, :],
                        start=True,
                        stop=True,
                    )
                    bc_sb = small.tile([P, SL], FP32, tag="bcs")
                    nc.vector.tensor_copy(bc_sb[0:64, :], bc[0:64, :])
                    if j == 0:
                        nc.vector.tensor_tensor(
                            ctx_sb[0:64, np_, :], ut[0:64, :], bc_sb[0:64, :], ALU.mult
                        )
                    else:
                        ctmp = small.tile([P, SL], FP32R, tag="ctmp")
                        nc.vector.tensor_tensor(
                            ctmp[0:64, :], ut[0:64, :], bc_sb[0:64, :], ALU.mult
                        )
                        # partition shift 0-63 -> 64-127 via SBUF-SBUF DMA
                        nc.sync.dma_start(ctx_sb[64:128, np_, :], ctmp[0:64, :])

            def prefetch_pair(pp_):
                vt = vatp.tile([P, NB, SQ, 130], FP32R, tag="vat", name=f"vt_{pp_}")
                for b in range(NB):
                    nc.sync.dma_start(vt[:, b], vf_blk[:, b, :, pp_, :])
                ktts = []
                for sb4 in range(NB):
                    kq = kttp.tile(
                        [P, SL], FP32R, tag="ktt", name=f"ktt_{pp_}_{sb4}"
                    )
                    nc.sync.dma_start(kq[:], kt_dr[:, pp_, ds(sb4 * SL, SL)])
                    ktts.append(kq)
                return vt, ktts

            tiles = {0: prefetch_pair(0)}
            norm_pend = None
            for p in range(PAIRS):
                utA = psAcc.tile([P, SL], FP32, tag="accA")
                utB = psAcc.tile([P, SL], FP32, tag="accB")
                vt, ktts = tiles.pop(p)
                # software pipeline: issue S^T/exp for chunk c+1 before the
                # U^T matmuls of chunk c, so the in-order PE never stalls on
                # ACT; the previous pair's normalize is likewise deferred into
                # this pair's stream so its PE broadcast never waits on DVE.
                pend = None
                for c in range(CH):
                    ktt = ktts[c // SQ][:, ts(c % SQ, P)]
                    ets = []
                    for j in range(2):
                        st = psA.tile([P, SL], FP32, tag="mm")
                        nc.tensor.matmul(
                            st[:],
                            ktt[ds(j * 64, 64), :],
                            qt_sb[ds(j * 64, 64), p, :],
                            start=True,
                            stop=True,
                        )
                        et = etp.tile([P, SL], FP32R, tag="et")
                        nc.scalar.activation(
                            out=et[:], in_=st[:], func=AF.Exp, scale=0.125
                        )
                        ets.append(et)
                    if c == 7 and norm_pend is not None:
                        emit_normalize(*norm_pend)
                        norm_pend = None
                    if c == 4 and p + 1 < PAIRS:
                        tiles[p + 1] = prefetch_pair(p + 1)
                    if pend is not None:
                        pc, pets, pv = pend
                        for j, ut in enumerate((utA, utB)):
                            nc.tensor.matmul(
                                ut[:65, :],
                                pv[:, ds(j * 65, 65)],
                                pets[j][:],
                                start=(pc == 0),
                                stop=False,
                            )
                    pend = (c, ets, vt[:, c // SQ, c % SQ, :])
                pc, pets, pv = pend
                for j, ut in enumerate((utA, utB)):
                    nc.tensor.matmul(
                        ut[:65, :],
                        pv[:, ds(j * 65, 65)],
                        pets[j][:],
                        start=False,
                        stop=True,
                    )
                norm_pend = (p, utA, utB)
            emit_normalize(*norm_pend)

            # ---- output projection + residual + LayerNorm ----
            out_sb = big.tile([P, SQ, D], FP32, tag="out")
            wo_sb = wpool.tile([P, KC, D], FP32R, tag="wrhs")
            for k in range(KC):
                nc.sync.dma_start(wo_sb[:, k, :], wo_d[ts(k, P), :])
            # i-outer so each row chunk's LayerNorm starts as soon as its
            # two 512-col halves are projected, instead of after all of them
            for i in range(SQ):
                for n in range(2):
                    pp = psA.tile([P, 512], FP32, tag="mm")
                    for p in range(PAIRS):
                        nc.tensor.matmul(
                            pp[:],
                            ctx_sb[:, p, ts(i, P)],
                            wo_sb[:, p, ds(n * 512, 512)],
                            start=(p == 0),
                            stop=(p == PAIRS - 1),
                        )
                    res = stream.tile([P, 512], FP32R, tag="res")
                    nc.sync.dma_start(res[:], xq_d[ts(i, P), ds(n * 512, 512)])
                    tmp = stream.tile([P, 512], FP32, tag="otmp")
                    nc.vector.tensor_tensor(tmp[:], pp[:], res[:], ALU.add)
                    nc.vector.tensor_tensor(
                        out_sb[:, i, ds(n * 512, 512)],
                        tmp[:],
                        bo_b[:, ds(n * 512, 512)],
                        ALU.add,
                    )
                row = out_sb[:, i, :]
                stats = small.tile([P, 2, 6], FP32, tag="stats")
                nc.vector.bn_stats(stats[:, 0, :], row[:, 0:512])
                nc.vector.bn_stats(stats[:, 1, :], row[:, 512:1024])
                mv = small.tile([P, 2], FP32, tag="mv")
                nc.vector.bn_aggr(mv[:], stats[:])
                std = small.tile([P, 1], FP32, tag="std")
                nc.scalar.activation(
                    out=std[:], in_=mv[:, 1:2], func=AF.Sqrt, bias=eps_t[:], scale=1.0
                )
                rstd = small.tile([P, 1], FP32, tag="rstd")
                nc.vector.reciprocal(out=rstd[:], in_=std[:])
                ytile = wide.tile([P, D], FP32, tag="y")
                nc.vector.tensor_scalar(
                    out=ytile[:],
                    in0=row,
                    scalar1=mv[:, 0:1],
                    scalar2=rstd[:],
                    op0=ALU.subtract,
                    op1=ALU.mult,
                )
                nc.vector.tensor_tensor(ytile[:], ytile[:], gam_b[:], ALU.mult)
                nc.vector.tensor_tensor(ytile[:], ytile[:], bet_b[:], ALU.add)
                nc.sync.dma_start(y_d[ts(i, P), :], ytile[:])

    nc.compile()
    return nc


def get_nc():
    if "nc" not in _NC_CACHE:
        _NC_CACHE["nc"] = build_nc()
    return _NC_CACHE["nc"]


def kernel(
    query,
    key,
    value,
    Wq,
    bq,
    Wk,
    bk,
    Wv,
    bv,
    Wo,
    bo,
    ln_gamma,
    ln_beta,
    _trace=False,
    _trace_cores=None,
):
    query = np.ascontiguousarray(np.asarray(query, dtype=np.float32))
    key = np.ascontiguousarray(np.asarray(key, dtype=np.float32))
    value = np.ascontiguousarray(np.asarray(value, dtype=np.float32))
    shared = {
        "wq": np.ascontiguousarray(np.asarray(Wq, np.float32)),
        "wk": np.ascontiguousarray(np.asarray(Wk, np.float32)),
        "wv": np.ascontiguousarray(np.asarray(Wv, np.float32)),
        "wo": np.ascontiguousarray(np.asarray(Wo, np.float32)),
        "bq": np.ascontiguousarray(np.asarray(bq, np.float32)),
        "bk": np.ascontiguousarray(np.asarray(bk, np.float32)),
        "bv": np.ascontiguousarray(np.asarray(bv, np.float32)),
        "bo": np.ascontiguousarray(np.asarray(bo, np.float32)),
        "gam": np.ascontiguousarray(np.asarray(ln_gamma, np.float32)),
        "bet": np.ascontiguousarray(np.asarray(ln_beta, np.float32)),
        "ident": np.eye(P, dtype=np.float32),
        "ones": np.ones((P, 64), dtype=np.float32),
    }
    in_maps = []
    for c in range(N_CORES):
        b, r = divmod(c, NB)
        rows = slice(r * SL, (r + 1) * SL)
        m = dict(shared)
        m["xq"] = np.ascontiguousarray(query[b, rows, :])
        m["xk"] = np.ascontiguousarray(key[b])
        m["xv"] = np.ascontiguousarray(value[b])
        in_maps.append(m)

    nc = get_nc()
    res = run_bass_kernel_spmd(
        nc,
        in_maps,
        list(range(N_CORES)),
        trace=_trace,
        trace_cores=_trace_cores,
    )
    out = np.empty((B, S, D), dtype=np.float32)
    for c in range(N_CORES):
        b, r = divmod(c, NB)
        out[b, r * SL : (r + 1) * SL, :] = res.results[c]["y"]
    if _trace:
        return out, res
    return out
